# revision 5
# baseline (speedup 1.0000x reference)
"""Trainium2 Bass kernel for nn_CausalGemAttention.

Reference computation (B=2, T=2048, C=1024, H=16, d=64):
    qkv = x @ w_attn + b_attn ; q,k,v = split(qkv)
    p = sign(sign(p_param)+0.5) * clamp(|p_param|, 1e-4, 1e3)
    vc = clip(|v + 5|, 1e-10); z = p*ln(vc); zmax = max_T(z); v' = exp(z - zmax)
    att = causal_softmax(q k^T / sqrt(d)); mean = att @ v'
    y = exp((zmax + ln(mean)) / p) - 5 ; out = y @ w_proj + b_proj

Sharding: 8 cores = 2 (batch) x 4 (head groups of 4 heads / 256 channels).
Each core computes qkv for its head group (contraction over full C), local
attention, and a partial projection (w_proj rows of its channels); host sums
the 4 partials per batch and adds b_proj.

All matmul operands use float32r (TF32-like: full-speed matmul at ~1e-4
relative precision); transform math (log/exp) stays fp32.
"""

import sys
sys.path.insert(0, "/opt/trn_rl_repo")

import numpy as np

import concourse.bacc as bacc
import concourse.tile as tile
from concourse import mybir
from concourse.bass_utils import run_bass_kernel_spmd

F32 = mybir.dt.float32
F32R = mybir.dt.float32r
AF = mybir.ActivationFunctionType
ALU = mybir.AluOpType
AX = mybir.AxisListType

B, T, C, H, D = 2, 2048, 1024, 16, 64
P = 128
CL = 256            # channels per core (4 heads x 64)
KC = C // P         # 8 contraction chunks for qkv
NQ = T // 512       # 4 query blocks of 512
NK = T // P         # 16 key tiles of 128
SHIFT = 5.0
P_MIN, P_MAX, V_MIN = 1e-4, 1e3, 1e-10
SM_SCALE = 1.0 / 8.0  # 1/sqrt(64)

_CACHE = {}


def _build():
    nc = bacc.Bacc("TRN2", target_bir_lowering=False, debug=False)

    xt_d = nc.dram_tensor("xt", [C, T], F32R, kind="ExternalInput")
    wq_d = nc.dram_tensor("wq", [C, CL], F32R, kind="ExternalInput")
    wk_d = nc.dram_tensor("wk", [C, CL], F32R, kind="ExternalInput")
    wv_d = nc.dram_tensor("wv", [C, CL], F32R, kind="ExternalInput")
    wp_d = nc.dram_tensor("wp", [CL, C], F32R, kind="ExternalInput")
    bq_d = nc.dram_tensor("bq", [P, 2], F32, kind="ExternalInput")
    bk_d = nc.dram_tensor("bk", [P, 2], F32, kind="ExternalInput")
    bv5_d = nc.dram_tensor("bv5", [P, 2], F32, kind="ExternalInput")   # b_v + SHIFT
    pp_d = nc.dram_tensor("pp", [P, 2], F32, kind="ExternalInput")     # p_param slice
    id_d = nc.dram_tensor("ident", [P, 64], F32R, kind="ExternalInput")
    mk_d = nc.dram_tensor("masks", [P, 4, 512], F32R, kind="ExternalInput")
    on_d = nc.dram_tensor("onesc", [P, NK], F32R, kind="ExternalInput")
    or_d = nc.dram_tensor("onesr", [1, 64], F32R, kind="ExternalInput")
    out_d = nc.dram_tensor("out_p", [T, C], F32, kind="ExternalOutput")

    with tile.TileContext(nc) as tc:
        with (
            tc.tile_pool(name="consts", bufs=1) as cp,
            tc.tile_pool(name="qk", bufs=1) as qkp,
            tc.tile_pool(name="vy", bufs=1) as vyp,
        ):
            ident = cp.tile([P, 64], F32R)
            masks = cp.tile([P, 4, 512], F32R)
            onesr = cp.tile([1, 64], F32R)
            bq_sb = cp.tile([P, 2], F32)
            bk_sb = cp.tile([P, 2], F32)
            bv5_sb = cp.tile([P, 2], F32)
            pp_sb = cp.tile([P, 2], F32)
            nc.sync.dma_start(ident[:], id_d[:])
            nc.sync.dma_start(masks[:], mk_d[:])
            nc.sync.dma_start(onesr[:], or_d[:])
            nc.sync.dma_start(bq_sb[:], bq_d[:])
            nc.sync.dma_start(bk_sb[:], bk_d[:])
            nc.sync.dma_start(bv5_sb[:], bv5_d[:])
            nc.sync.dma_start(pp_sb[:], pp_d[:])

            # p = sign(sign(pp)+0.5) * clamp(|pp|, P_MIN, P_MAX); ip = 1/p
            sgn = cp.tile([P, 2], F32)
            ab = cp.tile([P, 2], F32)
            p_sb = cp.tile([P, 2], F32)
            ip_sb = cp.tile([P, 2], F32)
            nc.scalar.activation(sgn[:], pp_sb[:], AF.Sign)
            nc.vector.tensor_scalar_add(sgn[:], sgn[:], 0.5)
            nc.scalar.activation(sgn[:], sgn[:], AF.Sign)
            nc.scalar.activation(ab[:], pp_sb[:], AF.Abs)
            nc.vector.tensor_scalar(ab[:], ab[:], float(P_MIN), float(P_MAX),
                                    ALU.max, ALU.min)
            nc.vector.tensor_tensor(p_sb[:], sgn[:], ab[:], ALU.mult)
            nc.vector.reciprocal(ip_sb[:], p_sb[:])

            negzmax = cp.tile([P, 2], F32)
            zmaxp = cp.tile([P, 2], F32)
            iph = cp.tile([64, 4], F32)    # per-head 1/p at partition base 0
            zmh = cp.tile([64, 4], F32)    # per-head zmax/p at partition base 0

            qT = qkp.tile([P, 2, T], F32R)   # q^T: [c%128, c//128, t]
            kT = qkp.tile([P, 2, T], F32R)
            vnat = vyp.tile([P, 4, NK, 65], F32R)  # [tk%128, head, tk//128, d|1]
            yT = vyp.tile([P, 2, T], F32R)
            wp_sb = vyp.tile([P, 2, C], F32R)
            nc.sync.dma_start(wp_sb[:], wp_d[:].rearrange("(c p) n -> p c n", p=P))

            # ---------------- Phase A: qkv + B: v transform + transposes -----
            with (
                tc.tile_pool(name="pA", bufs=1) as pA,
                tc.tile_pool(name="pB", bufs=1) as pB,
                tc.tile_pool(name="psA", bufs=4, space="PSUM") as psA,
                tc.tile_pool(name="psT", bufs=2, space="PSUM") as psT,
            ):
                xt_sb = pA.tile([P, KC, T], F32R)
                wq_sb = pA.tile([P, KC, CL], F32R)
                wk_sb = pA.tile([P, KC, CL], F32R)
                wv_sb = pA.tile([P, KC, CL], F32R)
                for kc in range(KC):
                    nc.sync.dma_start(xt_sb[:, kc, :], xt_d[kc * P:(kc + 1) * P, :])
                nc.sync.dma_start(wq_sb[:], wq_d[:].rearrange("(a p) m -> p a m", p=P))
                nc.sync.dma_start(wk_sb[:], wk_d[:].rearrange("(a p) m -> p a m", p=P))
                nc.sync.dma_start(wv_sb[:], wv_d[:].rearrange("(a p) m -> p a m", p=P))

                vT = pB.tile([P, 2, T], F32)
                vpT = pB.tile([P, 2, T], F32R)

                for m in range(2):           # channel chunk (128 of 256)
                    for nt in range(NQ):     # t block of 512
                        for wsb, kind in ((wq_sb, "q"), (wk_sb, "k"), (wv_sb, "v")):
                            ps = psA.tile([P, 512], F32, tag="ev")
                            for kc in range(KC):
                                nc.tensor.matmul(
                                    ps[:],
                                    wsb[:, kc, m * P:(m + 1) * P],
                                    xt_sb[:, kc, nt * 512:(nt + 1) * 512],
                                    start=(kc == 0), stop=(kc == KC - 1),
                                )
                            tsl = slice(nt * 512, (nt + 1) * 512)
                            if kind == "q":
                                nc.vector.tensor_scalar_add(
                                    qT[:, m, tsl], ps[:], bq_sb[:, m:m + 1])
                            elif kind == "k":
                                nc.vector.tensor_scalar_add(
                                    kT[:, m, tsl], ps[:], bk_sb[:, m:m + 1])
                            else:
                                # |v + b + SHIFT| directly out of PSUM
                                nc.scalar.activation(
                                    vT[:, m, tsl], ps[:], AF.Abs,
                                    bias=bv5_sb[:, m:m + 1])

                # transform: z = p*ln(clip(vc)), zmax, v' = exp(z - zmax)
                for m in range(2):
                    nc.vector.tensor_scalar_max(vT[:, m, :], vT[:, m, :], float(V_MIN))
                    nc.scalar.activation(vT[:, m, :], vT[:, m, :], AF.Ln)
                    nc.vector.tensor_scalar_mul(vT[:, m, :], vT[:, m, :],
                                                p_sb[:, m:m + 1])
                    nc.vector.tensor_reduce(negzmax[:, m:m + 1], vT[:, m, :], AX.X,
                                            op=ALU.max, negate=True)
                    nc.vector.scalar_tensor_tensor(
                        zmaxp[:, m:m + 1], negzmax[:, m:m + 1], -1.0,
                        ip_sb[:, m:m + 1], ALU.mult, ALU.mult)
                    nc.scalar.activation(vpT[:, m, :], vT[:, m, :], AF.Exp,
                                         bias=negzmax[:, m:m + 1])

                # per-head constants at partition base 0 (for ACT scale/bias APs)
                for h in range(4):
                    base, ch = 64 * (h % 2), h // 2
                    nc.sync.dma_start(iph[:, h:h + 1],
                                      ip_sb[base:base + 64, ch:ch + 1])
                    nc.sync.dma_start(zmh[:, h:h + 1],
                                      zmaxp[base:base + 64, ch:ch + 1])
                    nc.sync.dma_start(vnat[:, h, :, 64], on_d[:])

                # transpose v'^T [d, tk] -> vnat [tk, d], 8 k-tiles per PSUM bank
                for h in range(4):
                    base, ch = 64 * (h % 2), h // 2
                    for half in range(2):
                        trp = psT.tile([P, 512], F32R)
                        for j in range(8):
                            kt = half * 8 + j
                            nc.tensor.transpose(
                                trp[:, j * 64:(j + 1) * 64],
                                vpT[base:base + 64, ch, kt * P:(kt + 1) * P],
                                ident[base:base + 64, :],
                            )
                        nc.vector.tensor_copy(
                            vnat[:, h, half * 8:(half + 1) * 8, 0:64],
                            trp[:].rearrange("p (a b) -> p a b", a=8),
                        )

            # ---------------- Phase C: attention, D: projection --------------
            with (
                tc.tile_pool(name="att", bufs=6) as att,
                tc.tile_pool(name="small", bufs=3) as sm,
                tc.tile_pool(name="outp", bufs=3) as op_,
                tc.tile_pool(name="psS", bufs=4, space="PSUM") as psS,
                tc.tile_pool(name="psV", bufs=2, space="PSUM") as psV,
                tc.tile_pool(name="psP", bufs=2, space="PSUM") as psP,
            ):
                for hp in range(2):
                    h0, h1 = 2 * hp, 2 * hp + 1
                    ch = hp
                    for qi in range(NQ):
                        nk = 4 * (qi + 1)
                        qsl = slice(qi * 512, (qi + 1) * 512)
                        pv = [psV.tile([65, 512], F32, tag="pv", name=f"pv{_i}") for _i in range(2)]
                        prev = None
                        for kt in range(nk):
                            ksl = slice(kt * P, (kt + 1) * P)
                            ptile = []
                            for i, base in enumerate((0, 64)):
                                s_ps = psS.tile([P, 512], F32, tag="s")
                                nc.tensor.matmul(
                                    s_ps[:],
                                    kT[base:base + 64, ch, ksl],
                                    qT[base:base + 64, ch, qsl],
                                    start=True, stop=True,
                                    tile_position=(base, 0),
                                )
                                pt = att.tile([P, 512], F32R, tag="pT")
                                nc.scalar.activation(pt[:], s_ps[:], AF.Exp,
                                                     scale=SM_SCALE)
                                j = kt - 4 * qi
                                if j >= 0:
                                    nc.vector.tensor_mul(pt[:], pt[:],
                                                         masks[:, j, :])
                                ptile.append(pt)
                            if prev is not None:
                                pkt, pp0, pp1 = prev
                                for i, ppt in enumerate((pp0, pp1)):
                                    nc.tensor.matmul(
                                        pv[i][:],
                                        vnat[:, (h0, h1)[i], pkt, :],
                                        ppt[:],
                                        start=(pkt == 0), stop=False,
                                        skip_group_check=True,
                                    )
                            prev = (kt, ptile[0], ptile[1])
                        pkt, pp0, pp1 = prev
                        for i, ppt in enumerate((pp0, pp1)):
                            nc.tensor.matmul(
                                pv[i][:], vnat[:, (h0, h1)[i], pkt, :], ppt[:],
                                start=(pkt == 0), stop=True,
                                skip_group_check=True,
                            )

                        # post: mean = num/den; y = exp((zmax + ln(mean))/p) - 5
                        for i, h in enumerate((h0, h1)):
                            base = 64 * (h % 2)
                            rd = sm.tile([1, 512], F32R, tag="rd")
                            with nc.allow_low_precision(
                                    reason="f32r recip of softmax denom"):
                                nc.vector.reciprocal(rd[:], pv[i][64:65, :])
                            bc = psS.tile([64, 512], F32, tag="s")
                            nc.tensor.matmul(bc[:], onesr[:], rd[:],
                                             start=True, stop=True)
                            me = sm.tile([64, 512], F32, tag="me")
                            nc.vector.tensor_copy(me[:], pv[i][0:64, :])
                            nc.vector.tensor_tensor(me[:], me[:], bc[:], ALU.mult)
                            nc.scalar.activation(me[:], me[:], AF.Ln)
                            yh = sm.tile([64, 512], F32R, tag="yh")
                            nc.scalar.activation(yh[:], me[:], AF.Exp,
                                                 scale=iph[:, h:h + 1],
                                                 bias=zmh[:, h:h + 1])
                            nc.vector.tensor_scalar_add(yh[:], yh[:], -SHIFT)
                            nc.sync.dma_start(yT[base:base + 64, ch, qsl], yh[:])

                # projection: out[tq, :] += yT.T @ wp   (partial over 256 chans)
                for tq in range(T // P):
                    po = op_.tile([P, C], F32, tag="po")
                    for nh in range(2):
                        pj = psP.tile([P, 512], F32, tag="pj")
                        for c in range(2):
                            nc.tensor.matmul(
                                pj[:],
                                yT[:, c, tq * P:(tq + 1) * P],
                                wp_sb[:, c, nh * 512:(nh + 1) * 512],
                                start=(c == 0), stop=(c == 1),
                            )
                        nc.scalar.activation(po[:, nh * 512:(nh + 1) * 512], pj[:],
                                             AF.Copy)
                    nc.sync.dma_start(out_d[tq * P:(tq + 1) * P, :], po[:])

    nc.finalize()
    return nc


def _host_inputs(x, w_attn, b_attn, w_proj, p_param):
    """Build the 8 per-core input dicts."""
    ident = np.concatenate([np.eye(64, dtype=np.float32)] * 2, axis=0)
    xx = np.arange(P, dtype=np.int64)[:, None]
    yy = np.arange(512, dtype=np.int64)[None, :]
    masks = np.stack(
        [(yy - xx - P * j >= 0).astype(np.float32) for j in range(4)], axis=1)
    onesc = np.ones((P, NK), dtype=np.float32)
    onesr = np.ones((1, 64), dtype=np.float32)

    xts = [np.ascontiguousarray(x[b].T) for b in range(B)]
    in_maps = []
    for core in range(8):
        b, hg = divmod(core, 4)
        cs = slice(hg * CL, (hg + 1) * CL)
        csC = slice(C + hg * CL, C + (hg + 1) * CL)
        cs2C = slice(2 * C + hg * CL, 2 * C + (hg + 1) * CL)
        in_maps.append({
            "xt": xts[b],
            "wq": np.ascontiguousarray(w_attn[:, cs]),
            "wk": np.ascontiguousarray(w_attn[:, csC]),
            "wv": np.ascontiguousarray(w_attn[:, cs2C]),
            "wp": np.ascontiguousarray(w_proj[cs, :]),
            "bq": np.ascontiguousarray(b_attn[cs].reshape(2, P).T),
            "bk": np.ascontiguousarray(b_attn[csC].reshape(2, P).T),
            "bv5": np.ascontiguousarray(
                (b_attn[cs2C] + SHIFT).reshape(2, P).T.astype(np.float32)),
            "pp": np.ascontiguousarray(p_param[cs].reshape(2, P).T),
            "ident": ident,
            "masks": masks,
            "onesc": onesc,
            "onesr": onesr,
        })
    return in_maps


def kernel(x, w_attn, b_attn, w_proj, b_proj, p_param, _trace=False):
    x = np.asarray(x, dtype=np.float32)
    w_attn = np.asarray(w_attn, dtype=np.float32)
    b_attn = np.asarray(b_attn, dtype=np.float32)
    w_proj = np.asarray(w_proj, dtype=np.float32)
    b_proj = np.asarray(b_proj, dtype=np.float32)
    p_param = np.asarray(p_param, dtype=np.float32)

    if "nc" not in _CACHE:
        _CACHE["nc"] = _build()
    nc = _CACHE["nc"]

    in_maps = _host_inputs(x, w_attn, b_attn, w_proj, p_param)
    res = run_bass_kernel_spmd(nc, in_maps, core_ids=list(range(8)),
                               trace=_trace)
    _CACHE["last_result"] = res

    out = np.zeros((B, T, C), dtype=np.float32)
    for core in range(8):
        b = core // 4
        out[b] += res.results[core]["out_p"]
    out += b_proj[None, None, :]
    return out


if __name__ == "__main__":
    rng = np.random.default_rng(0)
    ins = {
        "x": rng.standard_normal((B, T, C), dtype=np.float32),
        "w_attn": (rng.standard_normal((C, 3 * C), dtype=np.float32) * 0.02),
        "b_attn": np.zeros(3 * C, np.float32),
        "w_proj": (rng.standard_normal((C, C), dtype=np.float32) * 0.02),
        "b_proj": np.zeros(C, np.float32),
        "p_param": np.ones(C, np.float32),
    }
    out = kernel(**ins)
    print("ran, out shape", out.shape, "finite:", np.isfinite(out).all())


# revision 6
# speedup vs baseline: 1.2218x; 1.2218x over previous
"""Trainium2 Bass kernel for nn_CausalGemAttention.

Reference computation (B=2, T=2048, C=1024, H=16, d=64):
    qkv = x @ w_attn + b_attn ; q,k,v = split(qkv)
    p = sign(sign(p_param)+0.5) * clamp(|p_param|, 1e-4, 1e3)
    vc = clip(|v + 5|, 1e-10); z = p*ln(vc); zmax = max_T(z); v' = exp(z - zmax)
    att = causal_softmax(q k^T / sqrt(d)); mean = att @ v'
    y = exp((zmax + ln(mean)) / p) - 5 ; out = y @ w_proj + b_proj

Sharding: 8 cores = 2 (batch) x 4 (head groups of 4 heads / 256 channels).
Each core computes qkv for its head group (contraction over full C), local
attention, and a partial projection (w_proj rows of its channels); host sums
the 4 partials per batch and adds b_proj.

Matmul operands are bf16 (fp32 PSUM accumulation); the log/exp transform and
softmax normalization run in fp32 (the softmax-denominator broadcast matmul
uses float32r to keep full precision there).
"""

import sys
sys.path.insert(0, "/opt/trn_rl_repo")

import numpy as np
import ml_dtypes

import concourse.bacc as bacc
import concourse.tile as tile
from concourse import mybir
from concourse.bass_utils import run_bass_kernel_spmd

F32 = mybir.dt.float32
F32R = mybir.dt.float32r
BF16 = mybir.dt.bfloat16
AF = mybir.ActivationFunctionType
ALU = mybir.AluOpType
AX = mybir.AxisListType

B, T, C, H, D = 2, 2048, 1024, 16, 64
P = 128
CL = 256            # channels per core (4 heads x 64)
KC = C // P         # 8 contraction chunks for qkv
NQ = T // 512       # 4 query blocks of 512
NK = T // P         # 16 key tiles of 128
SHIFT = 5.0
P_MIN, P_MAX, V_MIN = 1e-4, 1e3, 1e-10
SM_SCALE = 1.0 / 8.0  # 1/sqrt(64)

_CACHE = {}


def _build():
    nc = bacc.Bacc("TRN2", target_bir_lowering=False, debug=False)

    xt_d = nc.dram_tensor("xt", [C, T], BF16, kind="ExternalInput")
    wq_d = nc.dram_tensor("wq", [C, CL], BF16, kind="ExternalInput")
    wk_d = nc.dram_tensor("wk", [C, CL], BF16, kind="ExternalInput")
    wv_d = nc.dram_tensor("wv", [C, CL], BF16, kind="ExternalInput")
    wp_d = nc.dram_tensor("wp", [CL, C], BF16, kind="ExternalInput")
    bq_d = nc.dram_tensor("bq", [P, 2], F32, kind="ExternalInput")
    bk_d = nc.dram_tensor("bk", [P, 2], F32, kind="ExternalInput")
    bv5_d = nc.dram_tensor("bv5", [P, 2], F32, kind="ExternalInput")   # b_v + SHIFT
    pp_d = nc.dram_tensor("pp", [P, 2], F32, kind="ExternalInput")     # p_param slice
    id_d = nc.dram_tensor("ident", [P, 64], BF16, kind="ExternalInput")
    mk_d = nc.dram_tensor("masks", [P, 4, 512], BF16, kind="ExternalInput")
    on_d = nc.dram_tensor("onesc", [P, NK], BF16, kind="ExternalInput")
    or_d = nc.dram_tensor("onesr", [1, 64], F32R, kind="ExternalInput")
    out_d = nc.dram_tensor("out_p", [T, C], F32, kind="ExternalOutput")

    with tile.TileContext(nc) as tc:
        with (
            tc.tile_pool(name="consts", bufs=1) as cp,
            tc.tile_pool(name="qk", bufs=1) as qkp,
            tc.tile_pool(name="vy", bufs=1) as vyp,
        ):
            ident = cp.tile([P, 64], BF16)
            masks = cp.tile([P, 4, 512], BF16)
            onesr = cp.tile([1, 64], F32R)
            bq_sb = cp.tile([P, 2], F32)
            bk_sb = cp.tile([P, 2], F32)
            bv5_sb = cp.tile([P, 2], F32)
            pp_sb = cp.tile([P, 2], F32)
            nc.sync.dma_start(ident[:], id_d[:])
            nc.sync.dma_start(masks[:], mk_d[:])
            nc.sync.dma_start(onesr[:], or_d[:])
            nc.sync.dma_start(bq_sb[:], bq_d[:])
            nc.sync.dma_start(bk_sb[:], bk_d[:])
            nc.sync.dma_start(bv5_sb[:], bv5_d[:])
            nc.sync.dma_start(pp_sb[:], pp_d[:])

            # p = sign(sign(pp)+0.5) * clamp(|pp|, P_MIN, P_MAX); ip = 1/p
            sgn = cp.tile([P, 2], F32)
            ab = cp.tile([P, 2], F32)
            p_sb = cp.tile([P, 2], F32)
            ip_sb = cp.tile([P, 2], F32)
            nc.scalar.activation(sgn[:], pp_sb[:], AF.Sign)
            nc.vector.tensor_scalar_add(sgn[:], sgn[:], 0.5)
            nc.scalar.activation(sgn[:], sgn[:], AF.Sign)
            nc.scalar.activation(ab[:], pp_sb[:], AF.Abs)
            nc.vector.tensor_scalar(ab[:], ab[:], float(P_MIN), float(P_MAX),
                                    ALU.max, ALU.min)
            nc.vector.tensor_tensor(p_sb[:], sgn[:], ab[:], ALU.mult)
            nc.vector.reciprocal(ip_sb[:], p_sb[:])

            negzmax = cp.tile([P, 2], F32)
            zmaxp = cp.tile([P, 2], F32)
            iph = cp.tile([64, 4], F32)    # per-head 1/p at partition base 0
            zmh = cp.tile([64, 4], F32)    # per-head zmax/p at partition base 0

            qT = qkp.tile([P, 2, T], BF16)   # q^T: [c%128, c//128, t]
            kT = qkp.tile([P, 2, T], BF16)
            vnat = vyp.tile([P, 4, NK, 65], BF16)  # [tk%128, head, tk//128, d|1]
            yT = vyp.tile([P, 2, T], BF16)
            wp_sb = vyp.tile([P, 2, C], BF16)
            nc.sync.dma_start(wp_sb[:], wp_d[:].rearrange("(c p) n -> p c n", p=P))

            # ---------------- Phase A: qkv;  B: v transform + transposes -----
            with (
                tc.tile_pool(name="pA", bufs=1) as pA,
                tc.tile_pool(name="pB", bufs=1) as pB,
                tc.tile_pool(name="psA", bufs=4, space="PSUM") as psA,
                tc.tile_pool(name="psT", bufs=2, space="PSUM") as psT,
            ):
                xt_sb = pA.tile([P, KC, T], BF16)
                wq_sb = pA.tile([P, KC, CL], BF16)
                wk_sb = pA.tile([P, KC, CL], BF16)
                wv_sb = pA.tile([P, KC, CL], BF16)
                for kc in range(KC):
                    nc.sync.dma_start(xt_sb[:, kc, :], xt_d[kc * P:(kc + 1) * P, :])
                nc.sync.dma_start(wq_sb[:], wq_d[:].rearrange("(a p) m -> p a m", p=P))
                nc.sync.dma_start(wk_sb[:], wk_d[:].rearrange("(a p) m -> p a m", p=P))
                nc.sync.dma_start(wv_sb[:], wv_d[:].rearrange("(a p) m -> p a m", p=P))

                vT = pB.tile([P, 2, T], F32)
                vpT = pB.tile([P, 2, T], BF16)

                for m in range(2):           # channel chunk (128 of 256)
                    for nt in range(NQ):     # t block of 512
                        for wsb, kind in ((wq_sb, "q"), (wk_sb, "k"), (wv_sb, "v")):
                            ps = psA.tile([P, 512], F32, tag="ev")
                            for kc in range(KC):
                                nc.tensor.matmul(
                                    ps[:],
                                    wsb[:, kc, m * P:(m + 1) * P],
                                    xt_sb[:, kc, nt * 512:(nt + 1) * 512],
                                    start=(kc == 0), stop=(kc == KC - 1),
                                )
                            tsl = slice(nt * 512, (nt + 1) * 512)
                            if kind == "q":
                                nc.vector.tensor_scalar_add(
                                    qT[:, m, tsl], ps[:], bq_sb[:, m:m + 1])
                            elif kind == "k":
                                nc.vector.tensor_scalar_add(
                                    kT[:, m, tsl], ps[:], bk_sb[:, m:m + 1])
                            else:
                                # |v + b + SHIFT| directly out of PSUM
                                nc.scalar.activation(
                                    vT[:, m, tsl], ps[:], AF.Abs,
                                    bias=bv5_sb[:, m:m + 1])

                # transform: z = p*ln(clip(vc)), zmax, v' = exp(z - zmax)
                for m in range(2):
                    nc.vector.tensor_scalar_max(vT[:, m, :], vT[:, m, :], float(V_MIN))
                    nc.scalar.activation(vT[:, m, :], vT[:, m, :], AF.Ln)
                    nc.vector.tensor_scalar_mul(vT[:, m, :], vT[:, m, :],
                                                p_sb[:, m:m + 1])
                    nc.vector.tensor_reduce(negzmax[:, m:m + 1], vT[:, m, :], AX.X,
                                            op=ALU.max, negate=True)
                    nc.vector.scalar_tensor_tensor(
                        zmaxp[:, m:m + 1], negzmax[:, m:m + 1], -1.0,
                        ip_sb[:, m:m + 1], ALU.mult, ALU.mult)
                    nc.scalar.activation(vpT[:, m, :], vT[:, m, :], AF.Exp,
                                         bias=negzmax[:, m:m + 1])

                # per-head constants at partition base 0 (for ACT scale/bias APs)
                for h in range(4):
                    base, ch = 64 * (h % 2), h // 2
                    nc.sync.dma_start(iph[:, h:h + 1],
                                      ip_sb[base:base + 64, ch:ch + 1])
                    nc.sync.dma_start(zmh[:, h:h + 1],
                                      zmaxp[base:base + 64, ch:ch + 1])
                    nc.sync.dma_start(vnat[:, h, :, 64], on_d[:])

                # transpose v'^T [d, tk] -> vnat [tk, d], 8 k-tiles per PSUM bank
                for h in range(4):
                    base, ch = 64 * (h % 2), h // 2
                    for half in range(2):
                        trp = psT.tile([P, 512], BF16)
                        for j in range(8):
                            kt = half * 8 + j
                            nc.tensor.transpose(
                                trp[:, j * 64:(j + 1) * 64],
                                vpT[base:base + 64, ch, kt * P:(kt + 1) * P],
                                ident[base:base + 64, :],
                            )
                        nc.vector.tensor_copy(
                            vnat[:, h, half * 8:(half + 1) * 8, 0:64],
                            trp[:].rearrange("p (a b) -> p a b", a=8),
                        )

            # ---------------- Phase C: attention ------------------------------
            with (
                tc.tile_pool(name="att", bufs=5) as att,
                tc.tile_pool(name="small", bufs=3) as sm,
                tc.tile_pool(name="psS", bufs=3, space="PSUM") as psS,
                tc.tile_pool(name="psV", bufs=2, space="PSUM") as psV,
            ):
                for hp in range(2):
                    h0, h1 = 2 * hp, 2 * hp + 1
                    ch = hp
                    for qi in range(NQ):
                        npair = 2 * (qi + 1)       # kt pairs (kt = 2a, 2a+1)
                        qsl = slice(qi * 512, (qi + 1) * 512)
                        pv = [psV.tile([65, 512], F32, tag="pv", name=f"pv{_i}")
                              for _i in range(2)]
                        prev = None
                        for a in range(npair):
                            ptile = []
                            for i, base in enumerate((0, 64)):
                                s_ps = psS.tile([P, 1024], F32, tag="s")
                                for half in range(2):
                                    kt = 2 * a + half
                                    ksl = slice(kt * P, (kt + 1) * P)
                                    nc.tensor.matmul(
                                        s_ps[:, half * 512:(half + 1) * 512],
                                        kT[base:base + 64, ch, ksl],
                                        qT[base:base + 64, ch, qsl],
                                        start=True, stop=True,
                                        skip_group_check=True,
                                    )
                                pt = att.tile([P, 1024], BF16, tag="pT")
                                nc.scalar.activation(pt[:], s_ps[:], AF.Exp,
                                                     scale=SM_SCALE)
                                j0 = 2 * a - 4 * qi
                                if j0 >= 0:   # diagonal band: mask pair j0, j0+1
                                    nc.vector.tensor_mul(
                                        pt[:].rearrange("p (a b) -> p a b", a=2),
                                        pt[:].rearrange("p (a b) -> p a b", a=2),
                                        masks[:, j0:j0 + 2, :])
                                ptile.append(pt)
                            if prev is not None:
                                pa_, pp0, pp1 = prev
                                for i, ppt in enumerate((pp0, pp1)):
                                    for half in range(2):
                                        kt = 2 * pa_ + half
                                        nc.tensor.matmul(
                                            pv[i][:],
                                            vnat[:, (h0, h1)[i], kt, :],
                                            ppt[:, half * 512:(half + 1) * 512],
                                            start=(kt == 0), stop=False,
                                            skip_group_check=True,
                                        )
                            prev = (a, ptile[0], ptile[1])
                        pa_, pp0, pp1 = prev
                        for i, ppt in enumerate((pp0, pp1)):
                            for half in range(2):
                                kt = 2 * pa_ + half
                                nc.tensor.matmul(
                                    pv[i][:],
                                    vnat[:, (h0, h1)[i], kt, :],
                                    ppt[:, half * 512:(half + 1) * 512],
                                    start=(kt == 0), stop=(half == 1),
                                    skip_group_check=True,
                                )

                        # post: mean = num/den; y = exp((zmax + ln(mean))/p) - 5
                        for i, h in enumerate((h0, h1)):
                            base = 64 * (h % 2)
                            rd = sm.tile([1, 512], F32R, tag="rd")
                            with nc.allow_low_precision(
                                    reason="f32r recip of softmax denom"):
                                nc.vector.reciprocal(rd[:], pv[i][64:65, :])
                            bc = psS.tile([64, 512], F32, tag="s")
                            nc.tensor.matmul(bc[:], onesr[:], rd[:],
                                             start=True, stop=True)
                            me = sm.tile([64, 512], F32, tag="me")
                            nc.vector.tensor_copy(me[:], pv[i][0:64, :])
                            nc.vector.tensor_tensor(me[:], me[:], bc[:], ALU.mult)
                            nc.scalar.activation(me[:], me[:], AF.Ln)
                            yh = sm.tile([64, 512], BF16, tag="yh")
                            nc.scalar.activation(yh[:], me[:], AF.Exp,
                                                 scale=iph[:, h:h + 1],
                                                 bias=zmh[:, h:h + 1])
                            nc.vector.tensor_scalar_add(yh[:], yh[:], -SHIFT)
                            nc.sync.dma_start(yT[base:base + 64, ch, qsl], yh[:])

            # ---------------- Phase D: projection -----------------------------
            with (
                tc.tile_pool(name="outp", bufs=3) as op_,
                tc.tile_pool(name="psP", bufs=4, space="PSUM") as psP,
            ):
                for tq in range(T // P):
                    po = op_.tile([P, C], F32, tag="po")
                    for nh in range(2):
                        pj = psP.tile([P, 512], F32, tag="pj")
                        for c in range(2):
                            nc.tensor.matmul(
                                pj[:],
                                yT[:, c, tq * P:(tq + 1) * P],
                                wp_sb[:, c, nh * 512:(nh + 1) * 512],
                                start=(c == 0), stop=(c == 1),
                            )
                        nc.scalar.activation(po[:, nh * 512:(nh + 1) * 512], pj[:],
                                             AF.Copy)
                    nc.sync.dma_start(out_d[tq * P:(tq + 1) * P, :], po[:])

    nc.finalize()
    return nc


def _host_inputs(x, w_attn, b_attn, w_proj, p_param):
    """Build the 8 per-core input dicts."""
    bf16 = ml_dtypes.bfloat16
    ident = np.concatenate([np.eye(64, dtype=np.float32)] * 2, axis=0).astype(bf16)
    xx = np.arange(P, dtype=np.int64)[:, None]
    yy = np.arange(512, dtype=np.int64)[None, :]
    masks = np.stack(
        [(yy - xx - P * j >= 0).astype(np.float32) for j in range(4)],
        axis=1).astype(bf16)
    onesc = np.ones((P, NK), dtype=bf16)
    onesr = np.ones((1, 64), dtype=np.float32)

    xts = [np.ascontiguousarray(x[b].T).astype(bf16) for b in range(B)]
    in_maps = []
    for core in range(8):
        b, hg = divmod(core, 4)
        cs = slice(hg * CL, (hg + 1) * CL)
        csC = slice(C + hg * CL, C + (hg + 1) * CL)
        cs2C = slice(2 * C + hg * CL, 2 * C + (hg + 1) * CL)
        in_maps.append({
            "xt": xts[b],
            "wq": np.ascontiguousarray(w_attn[:, cs]).astype(bf16),
            "wk": np.ascontiguousarray(w_attn[:, csC]).astype(bf16),
            "wv": np.ascontiguousarray(w_attn[:, cs2C]).astype(bf16),
            "wp": np.ascontiguousarray(w_proj[cs, :]).astype(bf16),
            "bq": np.ascontiguousarray(b_attn[cs].reshape(2, P).T),
            "bk": np.ascontiguousarray(b_attn[csC].reshape(2, P).T),
            "bv5": np.ascontiguousarray(
                (b_attn[cs2C] + SHIFT).reshape(2, P).T.astype(np.float32)),
            "pp": np.ascontiguousarray(p_param[cs].reshape(2, P).T),
            "ident": ident,
            "masks": masks,
            "onesc": onesc,
            "onesr": onesr,
        })
    return in_maps


def kernel(x, w_attn, b_attn, w_proj, b_proj, p_param, _trace=False):
    x = np.asarray(x, dtype=np.float32)
    w_attn = np.asarray(w_attn, dtype=np.float32)
    b_attn = np.asarray(b_attn, dtype=np.float32)
    w_proj = np.asarray(w_proj, dtype=np.float32)
    b_proj = np.asarray(b_proj, dtype=np.float32)
    p_param = np.asarray(p_param, dtype=np.float32)

    if "nc" not in _CACHE:
        _CACHE["nc"] = _build()
    nc = _CACHE["nc"]

    in_maps = _host_inputs(x, w_attn, b_attn, w_proj, p_param)
    res = run_bass_kernel_spmd(nc, in_maps, core_ids=list(range(8)),
                               trace=_trace)
    _CACHE["last_result"] = res

    out = np.zeros((B, T, C), dtype=np.float32)
    for core in range(8):
        b = core // 4
        out[b] += res.results[core]["out_p"]
    out += b_proj[None, None, :]
    return out


if __name__ == "__main__":
    rng = np.random.default_rng(0)
    ins = {
        "x": rng.standard_normal((B, T, C), dtype=np.float32),
        "w_attn": (rng.standard_normal((C, 3 * C), dtype=np.float32) * 0.02),
        "b_attn": np.zeros(3 * C, np.float32),
        "w_proj": (rng.standard_normal((C, C), dtype=np.float32) * 0.02),
        "b_proj": np.zeros(C, np.float32),
        "p_param": np.ones(C, np.float32),
    }
    out = kernel(**ins)
    print("ran, out shape", out.shape, "finite:", np.isfinite(out).all())


# revision 9
# speedup vs baseline: 1.6743x; 1.3703x over previous
"""Trainium2 Bass kernel for nn_CausalGemAttention.

Reference computation (B=2, T=2048, C=1024, H=16, d=64):
    qkv = x @ w_attn + b_attn ; q,k,v = split(qkv)
    p = sign(sign(p_param)+0.5) * clamp(|p_param|, 1e-4, 1e3)
    vc = clip(|v + 5|, 1e-10); z = p*ln(vc); zmax = max_T(z); v' = exp(z - zmax)
    att = causal_softmax(q k^T / sqrt(d)); mean = att @ v'
    y = exp((zmax + ln(mean)) / p) - 5 ; out = y @ w_proj + b_proj

Sharding: 8 cores = 2 (batch) x 4 (head groups of 4 heads / 256 channels).
Each core computes qkv for its head group (contraction over full C), local
attention, and a partial projection (w_proj rows of its channels); host sums
the 4 partials per batch and adds b_proj.

Matmul operands are bf16 with fp32 PSUM accumulation.  To keep bf16 rounding
out of the softmax average (the +5 shift amplifies relative error of the
mean ~8x), v' is centered per channel before the PV matmul:
    mean = num''/den + cmid   with   v'' = v' - cmid
cmid is carried in fp32 and re-added exactly.  When p == 1 (the shipped
configuration) the final transform reduces to y = exp(zmax)*mean - 5 and is
computed without any per-tile ln/exp; a general-p fallback path exists.
"""

import sys
sys.path.insert(0, "/opt/trn_rl_repo")

import numpy as np
import ml_dtypes

import concourse.bacc as bacc
import concourse.tile as tile
from concourse import mybir
from concourse.bass_utils import run_bass_kernel_spmd

F32 = mybir.dt.float32
F32R = mybir.dt.float32r
BF16 = mybir.dt.bfloat16
AF = mybir.ActivationFunctionType
ALU = mybir.AluOpType
AX = mybir.AxisListType

B, T, C, H, D = 2, 2048, 1024, 16, 64
P = 128
CL = 256            # channels per core (4 heads x 64)
KC = C // P         # 8 contraction chunks for qkv
NQ = T // 512       # 4 query blocks of 512
NK = T // P         # 16 key tiles of 128
SHIFT = 5.0
P_MIN, P_MAX, V_MIN = 1e-4, 1e3, 1e-10
SM_SCALE = 1.0 / 8.0  # 1/sqrt(64)

_CACHE = {}


def _build(fast_p1):
    nc = bacc.Bacc("TRN2", target_bir_lowering=False, debug=False)

    xt_d = nc.dram_tensor("xt", [C, T], BF16, kind="ExternalInput")
    wq_d = nc.dram_tensor("wq", [C, CL], BF16, kind="ExternalInput")
    wk_d = nc.dram_tensor("wk", [C, CL], BF16, kind="ExternalInput")
    wv_d = nc.dram_tensor("wv", [C, CL], BF16, kind="ExternalInput")
    wp_d = nc.dram_tensor("wp", [CL, C], BF16, kind="ExternalInput")
    bq_d = nc.dram_tensor("bq", [P, 2], F32, kind="ExternalInput")
    bk_d = nc.dram_tensor("bk", [P, 2], F32, kind="ExternalInput")
    bv5_d = nc.dram_tensor("bv5", [P, 2], F32, kind="ExternalInput")   # b_v + SHIFT
    pp_d = nc.dram_tensor("pp", [P, 2], F32, kind="ExternalInput")     # p_param slice
    id_d = nc.dram_tensor("ident", [P, 64], BF16, kind="ExternalInput")
    mk_d = nc.dram_tensor("masks", [P, 4, 512], BF16, kind="ExternalInput")
    on_d = nc.dram_tensor("onesc", [P, NK], BF16, kind="ExternalInput")
    or_d = nc.dram_tensor("onesr", [1, 64], F32R, kind="ExternalInput")
    out_d = nc.dram_tensor("out_p", [T, C], F32, kind="ExternalOutput")

    with tile.TileContext(nc) as tc:
        with (
            tc.tile_pool(name="consts", bufs=1) as cp,
            tc.tile_pool(name="qk", bufs=1) as qkp,
            tc.tile_pool(name="vy", bufs=1) as vyp,
        ):
            ident = cp.tile([P, 64], BF16)
            masks = cp.tile([P, 4, 512], BF16)
            onesr = cp.tile([1, 64], F32R)
            bq_sb = cp.tile([P, 2], F32)
            bk_sb = cp.tile([P, 2], F32)
            bv5_sb = cp.tile([P, 2], F32)
            pp_sb = cp.tile([P, 2], F32)
            nc.sync.dma_start(ident[:], id_d[:])
            nc.sync.dma_start(masks[:], mk_d[:])
            nc.sync.dma_start(onesr[:], or_d[:])
            nc.sync.dma_start(bq_sb[:], bq_d[:])
            nc.sync.dma_start(bk_sb[:], bk_d[:])
            nc.sync.dma_start(bv5_sb[:], bv5_d[:])
            nc.sync.dma_start(pp_sb[:], pp_d[:])

            # p = sign(sign(pp)+0.5) * clamp(|pp|, P_MIN, P_MAX); ip = 1/p
            sgn = cp.tile([P, 2], F32)
            ab = cp.tile([P, 2], F32)
            p_sb = cp.tile([P, 2], F32)
            ip_sb = cp.tile([P, 2], F32)
            nc.scalar.activation(sgn[:], pp_sb[:], AF.Sign)
            nc.vector.tensor_scalar_add(sgn[:], sgn[:], 0.5)
            nc.scalar.activation(sgn[:], sgn[:], AF.Sign)
            nc.scalar.activation(ab[:], pp_sb[:], AF.Abs)
            nc.vector.tensor_scalar(ab[:], ab[:], float(P_MIN), float(P_MAX),
                                    ALU.max, ALU.min)
            nc.vector.tensor_tensor(p_sb[:], sgn[:], ab[:], ALU.mult)
            nc.vector.reciprocal(ip_sb[:], p_sb[:])

            negzmax = cp.tile([P, 2], F32)
            zmin_sb = cp.tile([P, 2], F32)
            zmaxp = cp.tile([P, 2], F32)
            cmid = cp.tile([P, 2], F32)     # 0.5*(1 + exp(zmin - zmax))
            ezp = cp.tile([P, 2], F32)      # exp(zmax)           (p==1 path)
            ecp5 = cp.tile([P, 2], F32)     # exp(zmax)*cmid - 5  (p==1 path)
            # per-head [64,1] copies at partition base 0 (ACT/DVE operand APs)
            iph = cp.tile([64, 4], F32)
            zmh = cp.tile([64, 4], F32)
            cmh = cp.tile([64, 4], F32)
            eph = cp.tile([64, 4], F32)
            ech = cp.tile([64, 4], F32)

            qT = qkp.tile([P, 2, T], BF16)   # q^T: [c%128, c//128, t]
            kT = qkp.tile([P, 2, T], BF16)
            vnat = vyp.tile([P, 4, NK, 65], BF16)  # [tk%128, head, tk//128, d|1]
            yT = vyp.tile([P, 2, T], BF16)
            wp_sb = vyp.tile([P, 2, C], BF16)
            nc.sync.dma_start(wp_sb[:], wp_d[:].rearrange("(c p) n -> p c n", p=P))

            # ---------------- Phase A: qkv;  B: v transform + transposes -----
            with (
                tc.tile_pool(name="pA", bufs=1) as pA,
                tc.tile_pool(name="pB", bufs=1) as pB,
                tc.tile_pool(name="psA", bufs=4, space="PSUM") as psA,
                tc.tile_pool(name="psT", bufs=2, space="PSUM") as psT,
            ):
                xt_sb = pA.tile([P, KC, T], BF16)
                wq_sb = pA.tile([P, KC, CL], BF16)
                wk_sb = pA.tile([P, KC, CL], BF16)
                wv_sb = pA.tile([P, KC, CL], BF16)
                for kc in range(KC):
                    nc.sync.dma_start(xt_sb[:, kc, :], xt_d[kc * P:(kc + 1) * P, :])
                nc.sync.dma_start(wq_sb[:], wq_d[:].rearrange("(a p) m -> p a m", p=P))
                nc.sync.dma_start(wk_sb[:], wk_d[:].rearrange("(a p) m -> p a m", p=P))
                nc.sync.dma_start(wv_sb[:], wv_d[:].rearrange("(a p) m -> p a m", p=P))

                vT = pB.tile([P, 2, T], F32)
                vpT = pB.tile([P, 2, T], BF16)

                for m in range(2):           # channel chunk (128 of 256)
                    for nt in range(NQ):     # t block of 512
                        for wsb, kind in ((wq_sb, "q"), (wk_sb, "k"), (wv_sb, "v")):
                            ps = psA.tile([P, 512], F32, tag="ev")
                            for kc in range(KC):
                                nc.tensor.matmul(
                                    ps[:],
                                    wsb[:, kc, m * P:(m + 1) * P],
                                    xt_sb[:, kc, nt * 512:(nt + 1) * 512],
                                    start=(kc == 0), stop=(kc == KC - 1),
                                )
                            tsl = slice(nt * 512, (nt + 1) * 512)
                            if kind == "q":
                                nc.vector.tensor_scalar_add(
                                    qT[:, m, tsl], ps[:], bq_sb[:, m:m + 1])
                            elif kind == "k":
                                nc.vector.tensor_scalar_add(
                                    kT[:, m, tsl], ps[:], bk_sb[:, m:m + 1])
                            else:
                                # |v + b + SHIFT| directly out of PSUM
                                nc.scalar.activation(
                                    vT[:, m, tsl], ps[:], AF.Abs,
                                    bias=bv5_sb[:, m:m + 1])

                # transform: z = p*ln(clip(vc)); zmax/zmin; v'' = e^(z-zmax)-cmid
                for m in range(2):
                    nc.vector.tensor_scalar_max(vT[:, m, :], vT[:, m, :],
                                                float(V_MIN))
                    nc.scalar.activation(vT[:, m, :], vT[:, m, :], AF.Ln)
                for m in range(2):
                    nc.vector.tensor_scalar_mul(vT[:, m, :], vT[:, m, :],
                                                p_sb[:, m:m + 1])
                    nc.vector.tensor_reduce(negzmax[:, m:m + 1], vT[:, m, :], AX.X,
                                            op=ALU.max, negate=True)
                    nc.vector.tensor_reduce(zmin_sb[:, m:m + 1], vT[:, m, :], AX.X,
                                            op=ALU.min)
                    nc.vector.scalar_tensor_tensor(
                        zmaxp[:, m:m + 1], negzmax[:, m:m + 1], -1.0,
                        ip_sb[:, m:m + 1], ALU.mult, ALU.mult)
                for m in range(2):
                    # cmid = 0.5*(1 + exp(zmin - zmax))
                    nc.scalar.activation(cmid[:, m:m + 1], zmin_sb[:, m:m + 1],
                                         AF.Exp, bias=negzmax[:, m:m + 1])
                    nc.vector.tensor_scalar(cmid[:, m:m + 1], cmid[:, m:m + 1],
                                            1.0, 0.5, ALU.add, ALU.mult)
                    # ezp = exp(zmax); ecp5 = ezp*cmid - 5
                    nc.scalar.activation(ezp[:, m:m + 1], negzmax[:, m:m + 1],
                                         AF.Exp, scale=-1.0)
                    nc.vector.scalar_tensor_tensor(
                        ecp5[:, m:m + 1], ezp[:, m:m + 1], 0.0,
                        cmid[:, m:m + 1], ALU.bypass, ALU.mult)
                    nc.vector.tensor_scalar_add(ecp5[:, m:m + 1], ecp5[:, m:m + 1],
                                                -SHIFT)
                    # v' (fp32, in place over z) then centered bf16 copy
                    nc.scalar.activation(vT[:, m, :], vT[:, m, :], AF.Exp,
                                         bias=negzmax[:, m:m + 1])
                    nc.vector.tensor_scalar_sub(vpT[:, m, :], vT[:, m, :],
                                                cmid[:, m:m + 1])

                # per-head constants at partition base 0
                for h in range(4):
                    base, ch = 64 * (h % 2), h // 2
                    sl = (slice(base, base + 64), slice(ch, ch + 1))
                    nc.sync.dma_start(iph[:, h:h + 1], ip_sb[sl])
                    nc.sync.dma_start(zmh[:, h:h + 1], zmaxp[sl])
                    nc.sync.dma_start(cmh[:, h:h + 1], cmid[sl])
                    nc.sync.dma_start(eph[:, h:h + 1], ezp[sl])
                    nc.sync.dma_start(ech[:, h:h + 1], ecp5[sl])
                    nc.sync.dma_start(vnat[:, h, :, 64], on_d[:])

                # transpose v''^T [d, tk] -> vnat [tk, d], 8 k-tiles per bank
                for h in range(4):
                    base, ch = 64 * (h % 2), h // 2
                    for half in range(2):
                        trp = psT.tile([P, 512], BF16)
                        for j in range(8):
                            kt = half * 8 + j
                            nc.tensor.transpose(
                                trp[:, j * 64:(j + 1) * 64],
                                vpT[base:base + 64, ch, kt * P:(kt + 1) * P],
                                ident[base:base + 64, :],
                            )
                        nc.vector.tensor_copy(
                            vnat[:, h, half * 8:(half + 1) * 8, 0:64],
                            trp[:].rearrange("p (a b) -> p a b", a=8),
                        )

            # ---------------- Phase C: attention ------------------------------
            with (
                tc.tile_pool(name="att", bufs=5) as att,
                tc.tile_pool(name="small", bufs=3) as sm,
                tc.tile_pool(name="psS", bufs=2, space="PSUM") as psS,
                tc.tile_pool(name="psV", bufs=3, space="PSUM") as psV,
            ):
                pending = []   # deferred post-chains (emitted mid next q-block)

                def post_chain(pv_t, h):
                    # mean'' = num''/den ; y = ezp*mean'' + (ezp*cmid - 5)
                    dcp = sm.tile([1, 512], F32, tag="dcp", name="dcp")
                    nc.vector.tensor_copy(dcp[:], pv_t[64:65, :])
                    rdf = sm.tile([1, 512], F32, tag="rdf", name="rdf")
                    nc.vector.reciprocal_approx_fast(rdf[:], dcp[:])
                    rd = sm.tile([1, 512], F32R, tag="rd", name="rd")
                    nc.vector.tensor_copy(rd[:], rdf[:])
                    bc = psV.tile([64, 512], F32, tag="bc", bufs=1, name="bc")
                    nc.tensor.matmul(bc[:], onesr[:], rd[:], start=True, stop=True)
                    me = sm.tile([64, 512], F32, tag="me", name="me")
                    nc.vector.tensor_copy(me[:], pv_t[0:64, :])
                    yh = sm.tile([64, 512], BF16, tag="yh", name="yh")
                    if fast_p1:
                        y1 = sm.tile([64, 512], F32, tag="y1", name="y1")
                        nc.vector.scalar_tensor_tensor(
                            y1[:], me[:], eph[:, h:h + 1], bc[:],
                            ALU.mult, ALU.mult)
                        nc.vector.tensor_scalar_add(yh[:], y1[:],
                                                    ech[:, h:h + 1])
                    else:
                        nc.vector.tensor_tensor(me[:], me[:], bc[:], ALU.mult)
                        nc.vector.tensor_scalar_add(me[:], me[:],
                                                    cmh[:, h:h + 1])
                        nc.scalar.activation(me[:], me[:], AF.Ln)
                        nc.scalar.activation(yh[:], me[:], AF.Exp,
                                             scale=iph[:, h:h + 1],
                                             bias=zmh[:, h:h + 1])
                        nc.vector.tensor_scalar_add(yh[:], yh[:], -SHIFT)
                    base, ch = 64 * (h % 2), h // 2
                    qi_ = post_chain_qi[0]
                    nc.sync.dma_start(
                        yT[base:base + 64, ch, qi_ * 512:(qi_ + 1) * 512], yh[:])

                post_chain_qi = [0]

                for hp in range(2):
                    h0, h1 = 2 * hp, 2 * hp + 1
                    ch = hp
                    for qi in range(NQ):
                        npair = 2 * (qi + 1)       # kt pairs (kt = 2a, 2a+1)
                        qsl = slice(qi * 512, (qi + 1) * 512)
                        pv = [psV.tile([65, 512], F32, tag="pv", name=f"pv{_i}")
                              for _i in range(2)]
                        prev = None
                        for a in range(npair):
                            ptile = []
                            # scores: 2 heads row-packed on the PE array
                            s_ps = [psS.tile([P, 1024], F32, tag="s",
                                             name=f"s{_i}") for _i in range(2)]
                            for half in range(2):
                                kt = 2 * a + half
                                ksl = slice(kt * P, (kt + 1) * P)
                                for i, base in enumerate((0, 64)):
                                    nc.tensor.matmul(
                                        s_ps[i][:, half * 512:(half + 1) * 512],
                                        kT[base:base + 64, ch, ksl],
                                        qT[base:base + 64, ch, qsl],
                                        start=True, stop=True,
                                        skip_group_check=True,
                                        tile_position=(base, 0),
                                    )
                            for i in range(2):
                                pt = att.tile([P, 1024], BF16, tag="pT",
                                              name="pt")
                                nc.scalar.activation(pt[:], s_ps[i][:], AF.Exp,
                                                     scale=SM_SCALE)
                                j0 = 2 * a - 4 * qi
                                if j0 >= 0:   # diagonal band: mask pair
                                    nc.vector.tensor_mul(
                                        pt[:],
                                        pt[:],
                                        masks[:, j0:j0 + 2, :].rearrange(
                                            "p a b -> p (a b)"))
                                ptile.append(pt)
                            if prev is not None:
                                pa_, pp0, pp1 = prev
                                for i, ppt in enumerate((pp0, pp1)):
                                    for half in range(2):
                                        kt = 2 * pa_ + half
                                        nc.tensor.matmul(
                                            pv[i][:],
                                            vnat[:, (h0, h1)[i], kt, :],
                                            ppt[:, half * 512:(half + 1) * 512],
                                            start=(kt == 0), stop=False,
                                            skip_group_check=True,
                                        )
                            if a == 1 and pending:
                                for fn in pending:
                                    fn()
                                pending.clear()
                            prev = (a, ptile[0], ptile[1])
                        pa_, pp0, pp1 = prev
                        for i, ppt in enumerate((pp0, pp1)):
                            for half in range(2):
                                kt = 2 * pa_ + half
                                nc.tensor.matmul(
                                    pv[i][:],
                                    vnat[:, (h0, h1)[i], kt, :],
                                    ppt[:, half * 512:(half + 1) * 512],
                                    start=(kt == 0), stop=(half == 1),
                                    skip_group_check=True,
                                )

                        def mk(pv_t, h, qi):
                            def fn():
                                post_chain_qi[0] = qi
                                post_chain(pv_t, h)
                            return fn
                        pending.append(mk(pv[0], h0, qi))
                        pending.append(mk(pv[1], h1, qi))
                for fn in pending:
                    fn()
                pending.clear()

            # ---------------- Phase D: projection -----------------------------
            with (
                tc.tile_pool(name="outp", bufs=3) as op_,
                tc.tile_pool(name="psP", bufs=4, space="PSUM") as psP,
            ):
                for tq in range(T // P):
                    po = op_.tile([P, C], F32, tag="po")
                    for nh in range(2):
                        pj = psP.tile([P, 512], F32, tag="pj")
                        for c in range(2):
                            nc.tensor.matmul(
                                pj[:],
                                yT[:, c, tq * P:(tq + 1) * P],
                                wp_sb[:, c, nh * 512:(nh + 1) * 512],
                                start=(c == 0), stop=(c == 1),
                            )
                        nc.scalar.activation(po[:, nh * 512:(nh + 1) * 512], pj[:],
                                             AF.Copy)
                    nc.sync.dma_start(out_d[tq * P:(tq + 1) * P, :], po[:])

    nc.finalize()
    return nc


def _host_inputs(x, w_attn, b_attn, w_proj, p_param):
    """Build the 8 per-core input dicts."""
    bf16 = ml_dtypes.bfloat16
    ident = np.concatenate([np.eye(64, dtype=np.float32)] * 2, axis=0).astype(bf16)
    xx = np.arange(P, dtype=np.int64)[:, None]
    yy = np.arange(512, dtype=np.int64)[None, :]
    masks = np.stack(
        [(yy - xx - P * j >= 0).astype(np.float32) for j in range(4)],
        axis=1).astype(bf16)
    onesc = np.ones((P, NK), dtype=bf16)
    onesr = np.ones((1, 64), dtype=np.float32)

    xts = [np.ascontiguousarray(x[b].T).astype(bf16) for b in range(B)]
    in_maps = []
    for core in range(8):
        b, hg = divmod(core, 4)
        cs = slice(hg * CL, (hg + 1) * CL)
        csC = slice(C + hg * CL, C + (hg + 1) * CL)
        cs2C = slice(2 * C + hg * CL, 2 * C + (hg + 1) * CL)
        in_maps.append({
            "xt": xts[b],
            "wq": np.ascontiguousarray(w_attn[:, cs]).astype(bf16),
            "wk": np.ascontiguousarray(w_attn[:, csC]).astype(bf16),
            "wv": np.ascontiguousarray(w_attn[:, cs2C]).astype(bf16),
            "wp": np.ascontiguousarray(w_proj[cs, :]).astype(bf16),
            "bq": np.ascontiguousarray(b_attn[cs].reshape(2, P).T),
            "bk": np.ascontiguousarray(b_attn[csC].reshape(2, P).T),
            "bv5": np.ascontiguousarray(
                (b_attn[cs2C] + SHIFT).reshape(2, P).T.astype(np.float32)),
            "pp": np.ascontiguousarray(p_param[cs].reshape(2, P).T),
            "ident": ident,
            "masks": masks,
            "onesc": onesc,
            "onesr": onesr,
        })
    return in_maps


def kernel(x, w_attn, b_attn, w_proj, b_proj, p_param, _trace=False):
    x = np.asarray(x, dtype=np.float32)
    w_attn = np.asarray(w_attn, dtype=np.float32)
    b_attn = np.asarray(b_attn, dtype=np.float32)
    w_proj = np.asarray(w_proj, dtype=np.float32)
    b_proj = np.asarray(b_proj, dtype=np.float32)
    p_param = np.asarray(p_param, dtype=np.float32)

    # p == 1 admits a cheaper final transform (no per-tile ln/exp)
    p_eff = np.sign(np.sign(p_param) + 0.5) * np.clip(np.abs(p_param),
                                                      P_MIN, P_MAX)
    fast_p1 = bool(np.all(p_eff == 1.0))

    key = ("nc", fast_p1)
    if key not in _CACHE:
        _CACHE[key] = _build(fast_p1)
    nc = _CACHE[key]

    in_maps = _host_inputs(x, w_attn, b_attn, w_proj, p_param)
    res = run_bass_kernel_spmd(nc, in_maps, core_ids=list(range(8)),
                               trace=_trace)
    _CACHE["last_result"] = res

    out = np.zeros((B, T, C), dtype=np.float32)
    for core in range(8):
        b = core // 4
        out[b] += res.results[core]["out_p"]
    out += b_proj[None, None, :]
    return out


if __name__ == "__main__":
    rng = np.random.default_rng(0)
    ins = {
        "x": rng.standard_normal((B, T, C), dtype=np.float32),
        "w_attn": (rng.standard_normal((C, 3 * C), dtype=np.float32) * 0.02),
        "b_attn": np.zeros(3 * C, np.float32),
        "w_proj": (rng.standard_normal((C, C), dtype=np.float32) * 0.02),
        "b_proj": np.zeros(C, np.float32),
        "p_param": np.ones(C, np.float32),
    }
    out = kernel(**ins)
    print("ran, out shape", out.shape, "finite:", np.isfinite(out).all())
